# revision 1
# baseline (speedup 1.0000x reference)
"""Causal self-attention kernel for 8 TRN2 NeuronCores.

Sharding: data-parallel over batch (B=8 -> 1 batch element per core).
Each core computes full 16-head causal attention for its batch element.
All matmuls run in bf16 with fp32 PSUM accumulation (~5.5e-3 rel err).

Per-core dataflow (L=1024, E=1024, H=16, D=64):
  XT  = x^T           host-pre-transposed bf16, loaded in l-slices
  QT  = Wq^T x^T + bq   [e, l] layout (stationary Wq blocks, moving XT)
  KT  = Wk^T x^T + bk   [e, l] layout
  V   = (x Wv + bv)|1   [l, h, 65] layout; the ones column makes the
                        attention matmul emit softmax denominators for free
  S^T = K Q^T           per (head, k-tile): [k,128, q<=1024] PSUM tiles,
                        contraction d=64; only causal tiles computed
  P^T = exp(S^T*scale)  ScalarE exp -> bf16 (scores bounded, no max sub);
                        diagonal tile masked by a 0/1 multiply on VectorE
  Yu  = [V|1]^T P^T     accumulated over k-tiles; row 64 = softmax sum s
  Y   = Yu[0:64] / s    s broadcast via DRAM round-trip + SWDGE bcast DMA +
                        approx reciprocal (last head pair: PE ones-matmul
                        broadcast instead, to kill the tail latency)
  out = Y^T.T Wo + bo   accumulation over e-tiles

QK-projection for e-tile et is interleaved with attention for heads
2et/2et+1 so ScalarE exp overlaps TensorE projection matmuls.

Measured (min of 3, core 0 neuron-profile): ~241-244 us/NEFF, TensorE
96.5% occupied within its span; L2 rel err vs fp32 reference ~5.5e-3.
"""

import os
import sys

sys.path.insert(0, "/opt/trn_rl_repo")

import numpy as np

import concourse.bass as bass
import concourse.mybir as mybir
import concourse.tile as tile
from concourse import bacc
from concourse.bass_utils import run_bass_kernel_spmd
f32 = mybir.dt.float32
f32r = mybir.dt.float32r
bf16 = mybir.dt.bfloat16
AF = mybir.ActivationFunctionType
OP = mybir.AluOpType

L = 1024
E = 1024
H = 16
D = 64
P = 128
NT = L // P  # 8 tiles along any 1024 dim
SCALE = 1.0 / np.sqrt(D)
KPHASE = int(os.environ.get("KPHASE", "5"))


def _build():
    nc = bacc.Bacc("TRN2", target_bir_lowering=False, debug=False, num_devices=8)
    x = nc.dram_tensor("x", [L, E], bf16, kind="ExternalInput").ap()
    wq = nc.dram_tensor("wq", [NT, P, NT, P], bf16, kind="ExternalInput").ap()
    wk = nc.dram_tensor("wk", [NT, P, NT, P], bf16, kind="ExternalInput").ap()
    wv = nc.dram_tensor("wv", [P, NT, E], bf16, kind="ExternalInput").ap()
    wo = nc.dram_tensor("wo", [P, NT, E], bf16, kind="ExternalInput").ap()
    bq = nc.dram_tensor("bq", [E], f32, kind="ExternalInput").ap()
    bk = nc.dram_tensor("bk", [E], f32, kind="ExternalInput").ap()
    bv = nc.dram_tensor("bv", [E], f32, kind="ExternalInput").ap()
    bo = nc.dram_tensor("bo", [E], f32, kind="ExternalInput").ap()
    xt_d = nc.dram_tensor("xt", [P, NT, L], bf16, kind="ExternalInput").ap()
    mask_d = nc.dram_tensor("mask01", [P, P], bf16, kind="ExternalInput").ap()
    out = nc.dram_tensor("out", [L, E], f32, kind="ExternalOutput").ap()
    s_dram = nc.dram_tensor("s_scratch", [H, L], f32, kind="Internal").ap()

    with tile.TileContext(nc) as tc:
        _body(nc, tc, wq, wk, wv, wo, bq, bk, bv, bo, out, s_dram,
              xt_d, mask_d)
    return nc


def _body(nc, tc, wq, wk, wv, wo, bq, bk, bv, bo, out, s_dram, xt_d, mask_d):
    from contextlib import ExitStack

    ctx = ExitStack()
    with ctx:
        consts = ctx.enter_context(tc.tile_pool(name="consts", bufs=1))
        qt_pool = ctx.enter_context(tc.tile_pool(name="qt_pool", bufs=1))
        kt_pool = ctx.enter_context(tc.tile_pool(name="kt_pool", bufs=1))
        v_pool = ctx.enter_context(tc.tile_pool(name="v_pool", bufs=1))
        y_pool = ctx.enter_context(tc.tile_pool(name="y_pool", bufs=1))
        sst_pool = ctx.enter_context(tc.tile_pool(name="sst_pool", bufs=4))
        pp = ctx.enter_context(tc.tile_pool(name="pp", bufs=2, space="PSUM"))
        sp = ctx.enter_context(tc.tile_pool(name="sp", bufs=2, space="PSUM"))
        yp = ctx.enter_context(tc.tile_pool(name="yp", bufs=2, space="PSUM"))
        wblk_pool = ctx.enter_context(tc.tile_pool(name="wblk_pool", bufs=4))
        pt_pool = ctx.enter_context(tc.tile_pool(name="pt_pool", bufs=4))
        osb_pool = ctx.enter_context(tc.tile_pool(name="osb_pool", bufs=3))

        mask01 = consts.tile([P, P], bf16)
        nc.sync.dma_start(out=mask01, in_=mask_d)
        ones_t = consts.tile([D + 1, P], bf16)
        nc.vector.memset(ones_t, 0.0)
        nc.vector.memset(ones_t[D : D + 1, :], 1.0)
        bq_sb = consts.tile([P, NT], f32)
        nc.sync.dma_start(out=bq_sb, in_=bq.rearrange("(et p) -> p et", p=P))
        bk_sb = consts.tile([P, NT], f32)
        nc.sync.dma_start(out=bk_sb, in_=bk.rearrange("(et p) -> p et", p=P))
        bv_bc = consts.tile([P, E], f32)
        nc.gpsimd.dma_start(
            out=bv_bc,
            in_=bass.AP(tensor=bv.tensor, offset=bv.offset, ap=[[0, P], [1, E]]),
        )
        bo_bc = consts.tile([P, E], f32)
        nc.gpsimd.dma_start(
            out=bo_bc,
            in_=bass.AP(tensor=bo.tensor, offset=bo.offset, ap=[[0, P], [1, E]]),
        )

        QT = qt_pool.tile([P, NT, L], bf16)  # [p, et, l] = Q^T[et*128+p, l]
        KT = kt_pool.tile([P, NT, L], bf16)
        V = v_pool.tile([P, NT, H, D + 1], bf16)  # [p(l), lt, h, d | ones]
        Y = y_pool.tile([P, NT, L], bf16)  # [p, et, l] = y^T[et*128+p, l]
        Ybc = Y

        nc.vector.memset(V[:, :, :, D : D + 1], 1.0)

        # ---- Phase 1: XT = x^T, pre-transposed on host ----
        if True:
            xt_pool = ctx.enter_context(tc.tile_pool(name="xt_pool", bufs=1))
            XT = xt_pool.tile([P, NT, L], bf16)  # [p, ct, l] = x^T[ct*128+p, l]
            xt_r = xt_d
            nc.sync.dma_start(out=XT[:, :, 0:256], in_=xt_r[:, :, 0:256])

            # ---- Phase 2a: V = x @ Wv + bv (V in natural [l, e] layout) ----
            with tc.tile_pool(name="wv_pool", bufs=1) as wvp:
              if KPHASE >= 2:
                  wv_blk = wvp.tile([P, NT, E], bf16)
                  nc.sync.dma_start(
                      out=wv_blk[:, :, 0:512], in_=wv[:, :, 0:512]
                  )
                  for ls in range(1, 4):
                      nc.sync.dma_start(
                          out=XT[:, :, ls * 256 : (ls + 1) * 256],
                          in_=xt_r[:, :, ls * 256 : (ls + 1) * 256],
                      )
                  nc.sync.dma_start(
                      out=wv_blk[:, :, 512:1024], in_=wv[:, :, 512:1024]
                  )
                  for ec in range(2):
                      for lt in range(NT):
                          ps = pp.tile([P, 512], f32, tag="pp")
                          for ct in range(NT):
                              nc.tensor.matmul(
                                  ps,
                                  XT[:, ct, lt * P : (lt + 1) * P],
                                  wv_blk[:, ct, ec * 512 : (ec + 1) * 512],
                                  start=(ct == 0),
                                  stop=(ct == NT - 1),
                              )
                          nc.vector.tensor_tensor(
                              out=V[:, lt, ec * 8 : (ec + 1) * 8, 0:D],
                              in0=ps.rearrange("p (h d) -> p h d", h=8),
                              in1=bv_bc[:, ec * 512 : (ec + 1) * 512].rearrange(
                                  "p (h d) -> p h d", h=8
                              ),
                              op=OP.add,
                          )

            # ---- Phase 2b+3: QT/KT per et, then attention for heads 2et, 2et+1 ----
            r_pool = ctx.enter_context(tc.tile_pool(name="r_pool", bufs=1))
            rh_pool = ctx.enter_context(tc.tile_pool(name="rh_pool", bufs=3))
            wo_pool = ctx.enter_context(tc.tile_pool(name="wo_pool", bufs=1))
            R = r_pool.tile([P, NT, L], f32)
            wo_r = wo_pool.tile([P, NT, E], bf16)
            for et in range(NT if KPHASE >= 3 else 0):
                for (w_dram, b_sb, dst) in ((wq, bq_sb, QT), (wk, bk_sb, KT)):
                    wqk_blk = wblk_pool.tile(
                        [P, NT, P], bf16, tag="wqkblk", name="wqk_blk"
                    )
                    nc.sync.dma_start(out=wqk_blk, in_=w_dram[et])
                    for lc in range(2):
                        ps = pp.tile([P, 512], f32, tag="pp")
                        for ct in range(NT):
                            nc.tensor.matmul(
                                ps,
                                wqk_blk[:, ct, :],
                                XT[:, ct, lc * 512 : (lc + 1) * 512],
                                start=(ct == 0),
                                stop=(ct == NT - 1),
                            )
                        nc.vector.tensor_scalar(
                            out=dst[:, et, lc * 512 : (lc + 1) * 512],
                            in0=ps,
                            scalar1=b_sb[:, et : et + 1],
                            scalar2=None,
                            op0=OP.add,
                        )

                last_pair = et == NT - 1
                for h in ((2 * et, 2 * et + 1) if KPHASE >= 4 else ()):
                    _attention_head(
                        nc, tc, h, QT, KT, V, Ybc, s_dram, sp, yp, pt_pool,
                        sst_pool, mask01, last_pair, ones_t, rh_pool, pp
                    )
                if KPHASE >= 5 and not last_pair:
                    for half in range(2):
                        hh = 2 * et + half
                        bsrc = bass.AP(
                            tensor=s_dram.tensor,
                            offset=s_dram[hh : hh + 1, :].offset,
                            ap=[[0, 64], [1, L]],
                        )
                        nc.gpsimd.dma_start(
                            out=R[half * 64 : (half + 1) * 64, et, :], in_=bsrc
                        )
                    nc.vector.reciprocal_approx_fast(out=R[:, et, :], in_=R[:, et, :])
                    for half in range(2):
                        rows = slice(half * 64, (half + 1) * 64)
                        nc.vector.tensor_tensor(
                            out=Ybc[rows, et, :],
                            in0=Y[rows, et, :],
                            in1=R[rows, et, :],
                            op=OP.mult,
                        )


        # ---- Phase 5: out = Y^T.T @ Wo + bo ----
        nc.sync.dma_start(out=wo_r, in_=wo)
        for lt in range(NT if KPHASE >= 5 else 0):
            for oc in range(2):
                ps = pp.tile([P, 512], f32, tag="pp", name="ps_out")
                for et in range(NT):
                    nc.tensor.matmul(
                        ps,
                        Ybc[:, et, lt * P : (lt + 1) * P],
                        wo_r[:, et, oc * 512 : (oc + 1) * 512],
                        start=(et == 0),
                        stop=(et == NT - 1),
                    )
                osb = osb_pool.tile([P, 512], f32)
                nc.vector.tensor_tensor(
                    out=osb, in0=ps, in1=bo_bc[:, oc * 512 : (oc + 1) * 512], op=OP.add
                )
                nc.sync.dma_start(
                    out=out[lt * P : (lt + 1) * P, oc * 512 : (oc + 1) * 512], in_=osb
                )


def _attention_head(nc, tc, h, QT, KT, V, Ybc, s_dram, sp, yp, pt_pool, sst_pool, mask01, last_pair, ones_t, rh_pool, pp):
    et = h // 2
    pb = (h % 2) * 64  # partition base of this head inside its e-tile

    yu = []
    for qc in range(2):
        t = yp.tile([D + 1, 512], f32, tag="yu", name=f"yu{qc}")
        yu.append(t)

    for kt in range(NT):
        qlen = L - kt * P
        st = sp.tile([P, L], f32, tag="st", name="st")
        # scores S^T[k, q] = K_h Q_h^T for q >= kt*128 (relative col = q - kt*128)
        for s0 in range(0, qlen, 512):
            n = min(512, qlen - s0)
            nc.tensor.matmul(
                st[:, s0 : s0 + n],
                KT[pb : pb + D, et, kt * P : (kt + 1) * P],
                QT[pb : pb + D, et, kt * P + s0 : kt * P + s0 + n],
                start=True,
                stop=True,
            )
        pt = pt_pool.tile([P, L], bf16, tag="pt", name="pt")
        nc.scalar.activation(
            out=pt[:, 0:qlen], in_=st[:, 0:qlen], func=AF.Exp, scale=float(SCALE)
        )
        # causal mask on the diagonal tile: zero where q < k (col j < row p)
        nc.vector.tensor_tensor(
            out=pt[:, 0:P], in0=pt[:, 0:P], in1=mask01, op=OP.mult
        )
        # AV: yu[qc] += [V|1]^T @ P^T
        for qc in range(2):
            lo = max(qc * 512, kt * P)
            hi = (qc + 1) * 512
            if lo >= hi:
                continue
            last_kt = min(NT - 1, (qc + 1) * 4 - 1)
            nc.tensor.matmul(
                yu[qc][:, lo - qc * 512 : hi - qc * 512],
                V[:, kt, h, :],
                pt[:, lo - kt * P : hi - kt * P],
                start=(kt == 0),
                stop=(kt == last_kt),
            )
            if kt == last_kt:
                _head_tail(
                    nc, h, qc, yu, Ybc, s_dram, sst_pool, last_pair, ones_t,
                    rh_pool, pp,
                )


def _head_tail(nc, h, qc, yu, Ybc, s_dram, sst_pool, last_pair, ones_t, rh_pool, pp):
    et = h // 2
    pb = (h % 2) * 64
    if True:
        cols = slice(qc * 512, (qc + 1) * 512)
        if last_pair:
            sstb = sst_pool.tile([D + 1, 512], bf16, tag="sstb", name="sstb")
            nc.vector.tensor_copy(out=sstb[D : D + 1, :], in_=yu[qc][D : D + 1, :])
            ps_bc = pp.tile([P, 512], f32, tag="pp", name="ps_bc")
            nc.tensor.matmul(
                ps_bc, ones_t[D : D + 1, :], sstb[D : D + 1, :],
                start=True, stop=True,
            )
            rh = rh_pool.tile([P, 512], f32, tag="rh", name="rh")
            nc.vector.reciprocal_approx_fast(out=rh, in_=ps_bc)
            nc.vector.tensor_tensor(
                out=Ybc[pb : pb + D, et, cols],
                in0=yu[qc][0:D, :],
                in1=rh[0:D, :],
                op=OP.mult,
            )
        else:
            sst = sst_pool.tile([D + 1, 512], f32, tag="sst", name="sst")
            nc.vector.tensor_copy(out=sst[D : D + 1, :], in_=yu[qc][D : D + 1, :])
            nc.sync.dma_start(out=s_dram[h : h + 1, cols], in_=sst[D : D + 1, :])
            nc.vector.tensor_copy(out=Ybc[pb : pb + D, et, cols], in_=yu[qc][0:D, :])


_COMPILED = None


def _get_compiled():
    global _COMPILED
    if _COMPILED is None:
        nc = _build()
        nc.compile()
        _COMPILED = nc
    return _COMPILED


def kernel(x, Wq, bq, Wk, bk, Wv, bv, Wo, bo, _trace=False):
    import ml_dtypes

    bfl = ml_dtypes.bfloat16
    nc = _get_compiled()
    x = np.ascontiguousarray(np.asarray(x, dtype=np.float32).astype(bfl))
    B = x.shape[0]
    assert B == 8 and x.shape[1] == L and x.shape[2] == E
    def _qk_layout(w):
        # [et, p, ct, e']: per-et contiguous [128, 8, 128] stationary blocks
        w = np.asarray(w, np.float32).astype(bfl)
        return np.ascontiguousarray(
            w.reshape(NT, P, NT, P).transpose(2, 1, 0, 3)
        )

    def _pct_layout(w):
        # [p, ct, e]: moving-operand blocks with contraction rows on partitions
        w = np.asarray(w, np.float32).astype(bfl)
        return np.ascontiguousarray(w.reshape(NT, P, E).transpose(1, 0, 2))

    common = {
        "wq": _qk_layout(Wq),
        "wk": _qk_layout(Wk),
        "wv": _pct_layout(Wv),
        "wo": _pct_layout(Wo),
        "bq": np.ascontiguousarray(np.asarray(bq, np.float32)),
        "bk": np.ascontiguousarray(np.asarray(bk, np.float32)),
        "bv": np.ascontiguousarray(np.asarray(bv, np.float32)),
        "bo": np.ascontiguousarray(np.asarray(bo, np.float32)),
    }
    common["mask01"] = np.tril(np.ones((P, P), np.float32)).T.astype(bfl)
    # xt[b]: [p, ct, l] with xt[b][p, ct, l] = x[b, l, ct*128+p]
    xt = np.ascontiguousarray(
        x.transpose(0, 2, 1).reshape(B, NT, P, L).transpose(0, 2, 1, 3)
    )
    in_maps = [dict(common, x=x[i], xt=xt[i]) for i in range(B)]
    res = run_bass_kernel_spmd(nc, in_maps, core_ids=list(range(8)), trace=_trace)
    outp = np.stack([res.results[i]["out"] for i in range(B)])
    if _trace:
        kernel.last_exec_time_ns = res.exec_time_ns
    return outp



# revision 2
# speedup vs baseline: 1.0050x; 1.0050x over previous
"""Causal self-attention kernel for 8 TRN2 NeuronCores.

Sharding: data-parallel over batch (B=8 -> 1 batch element per core).
Each core computes full 16-head causal attention for its batch element.
All matmuls run in bf16 with fp32 PSUM accumulation.

Per-core dataflow (L=1024, E=1024, H=16, D=64):
  XT  = x^T           host-pre-transposed bf16, one contiguous DMA
  QT  = Wq^T x^T + bq   [e, l] layout (stationary Wq blocks, moving XT)
  KT  = Wk^T x^T + bk   [e, l] layout
  V   = (x Wv + bv)|1   [l, h, 65] layout; the ones column makes the
                        attention matmul emit softmax denominators for free
  S^T = K Q^T           per (head, qc, kt): [k=128, chunk<=512] PSUM tiles,
                        contraction d=64; heads 2et/2et+1 issued back-to-back
                        as PE row-tiles (partitions 0-63 / 64-127) so the two
                        64-contraction matmuls execute concurrently
  P^T = exp(S^T*scale)  ScalarE exp -> bf16; diagonal chunk masked by a
                        0/1 multiply on VectorE
  Yu  = [V|1]^T P^T     accumulated per qc phase over kt; row 64 = softmax
                        denominator s
  Y   = Yu[0:64] / s    s broadcast via DRAM round-trip (gpsimd queue) +
                        approx reciprocal; last head pair uses a PE
                        ones-matmul broadcast instead to kill tail latency
  out = Y^T.T Wo + bo   accumulation over e-tiles, et=7 last per PSUM group

QK-projection matmuls for et+1 are interleaved between attention steps of
head pair (2et, 2et+1) so ScalarE exp stays off the TensorE critical path.
Attention is qc-phased (q columns 0-511 fully, then 512-1023) so only two
yu PSUM banks are live at a time, freeing banks for double-buffered
row-tiled score pairs.
"""

import os
import sys

sys.path.insert(0, "/opt/trn_rl_repo")

import numpy as np

import concourse.bass as bass
import concourse.mybir as mybir
import concourse.tile as tile
from concourse import bacc
from concourse.bass_utils import run_bass_kernel_spmd
f32 = mybir.dt.float32
f32r = mybir.dt.float32r
bf16 = mybir.dt.bfloat16
AF = mybir.ActivationFunctionType
OP = mybir.AluOpType

L = 1024
E = 1024
H = 16
D = 64
P = 128
NT = L // P  # 8 tiles along any 1024 dim
SCALE = 1.0 / np.sqrt(D)


def _build():
    nc = bacc.Bacc("TRN2", target_bir_lowering=False, debug=False, num_devices=8)
    x = nc.dram_tensor("x", [L, E], bf16, kind="ExternalInput").ap()
    wq = nc.dram_tensor("wq", [NT, P, NT, P], bf16, kind="ExternalInput").ap()
    wk = nc.dram_tensor("wk", [NT, P, NT, P], bf16, kind="ExternalInput").ap()
    wv = nc.dram_tensor("wv", [P, NT, E], bf16, kind="ExternalInput").ap()
    wo = nc.dram_tensor("wo", [P, NT, E], bf16, kind="ExternalInput").ap()
    bq = nc.dram_tensor("bq", [E], f32, kind="ExternalInput").ap()
    bk = nc.dram_tensor("bk", [E], f32, kind="ExternalInput").ap()
    bv = nc.dram_tensor("bv", [E], f32, kind="ExternalInput").ap()
    bo = nc.dram_tensor("bo", [E], f32, kind="ExternalInput").ap()
    xt_d = nc.dram_tensor("xt", [P, NT, L], bf16, kind="ExternalInput").ap()
    mask_d = nc.dram_tensor("mask01", [P, P], bf16, kind="ExternalInput").ap()
    out = nc.dram_tensor("out", [L, E], f32, kind="ExternalOutput").ap()
    s_dram = nc.dram_tensor("s_scratch", [H, L], f32, kind="Internal").ap()

    with tile.TileContext(nc) as tc:
        _body(nc, tc, wq, wk, wv, wo, bq, bk, bv, bo, out, s_dram,
              xt_d, mask_d)
    return nc


def _body(nc, tc, wq, wk, wv, wo, bq, bk, bv, bo, out, s_dram, xt_d, mask_d):
    from contextlib import ExitStack

    ctx = ExitStack()
    with ctx:
        consts = ctx.enter_context(tc.tile_pool(name="consts", bufs=1))
        qt_pool = ctx.enter_context(tc.tile_pool(name="qt_pool", bufs=1))
        kt_pool = ctx.enter_context(tc.tile_pool(name="kt_pool", bufs=1))
        v_pool = ctx.enter_context(tc.tile_pool(name="v_pool", bufs=1))
        y_pool = ctx.enter_context(tc.tile_pool(name="y_pool", bufs=1))
        xt_pool = ctx.enter_context(tc.tile_pool(name="xt_pool", bufs=1))
        wv_pool = ctx.enter_context(tc.tile_pool(name="wv_pool", bufs=1))
        wo_pool = ctx.enter_context(tc.tile_pool(name="wo_pool", bufs=1))
        r_pool = ctx.enter_context(tc.tile_pool(name="r_pool", bufs=1))
        sst_pool = ctx.enter_context(tc.tile_pool(name="sst_pool", bufs=4))
        rh_pool = ctx.enter_context(tc.tile_pool(name="rh_pool", bufs=3))
        wblk_pool = ctx.enter_context(tc.tile_pool(name="wblk_pool", bufs=4))
        pt_pool = ctx.enter_context(tc.tile_pool(name="pt_pool", bufs=6))
        osb_pool = ctx.enter_context(tc.tile_pool(name="osb_pool", bufs=3))
        pp = ctx.enter_context(tc.tile_pool(name="pp", bufs=2, space="PSUM"))
        sp = ctx.enter_context(tc.tile_pool(name="sp", bufs=4, space="PSUM"))
        yp = ctx.enter_context(tc.tile_pool(name="yp", bufs=2, space="PSUM"))

        # ---- constants (small DMAs on the scalar queue; broadcasts on gpsimd) ----
        mask01 = consts.tile([P, P], bf16)
        nc.scalar.dma_start(out=mask01, in_=mask_d)
        bq_sb = consts.tile([P, NT], f32)
        nc.scalar.dma_start(out=bq_sb, in_=bq.rearrange("(et p) -> p et", p=P))
        bk_sb = consts.tile([P, NT], f32)
        nc.scalar.dma_start(out=bk_sb, in_=bk.rearrange("(et p) -> p et", p=P))
        bv_bc = consts.tile([P, E], f32)
        nc.gpsimd.dma_start(
            out=bv_bc,
            in_=bass.AP(tensor=bv.tensor, offset=bv.offset, ap=[[0, P], [1, E]]),
        )
        bo_bc = consts.tile([P, E], f32)
        nc.gpsimd.dma_start(
            out=bo_bc,
            in_=bass.AP(tensor=bo.tensor, offset=bo.offset, ap=[[0, P], [1, E]]),
        )
        ones_t = consts.tile([D + 1, P], bf16)
        nc.vector.memset(ones_t, 0.0)
        nc.vector.memset(ones_t[D : D + 1, :], 1.0)

        QT = qt_pool.tile([P, NT, L], bf16)  # [p, et, l] = Q^T[et*128+p, l]
        KT = kt_pool.tile([P, NT, L], bf16)
        V = v_pool.tile([P, NT, H, D + 1], bf16)  # [p(l), lt, h, d | ones]
        Y = y_pool.tile([P, NT, L], bf16)  # [p, et, l] = y^T[et*128+p, l]
        Ybc = Y
        XT = xt_pool.tile([P, NT, L], bf16)  # [p, ct, l] = x^T[ct*128+p, l]
        wv_blk = wv_pool.tile([P, NT, E], bf16)
        wo_r = wo_pool.tile([P, NT, E], bf16)
        R = r_pool.tile([P, NT, L], f32)

        nc.vector.memset(V[:, :, :, D : D + 1], 1.0)

        # ---- bulk input DMAs ----
        # sync queue: wq/wk blocks (per-et prefetch) + XT (contiguous per
        # partition -> cheap descriptors). gpsimd queue: wv, wo (early!).
        wqk_blks = {}

        def prefetch_wqk(et):
            blks = []
            for w_dram in (wq, wk):
                blk = wblk_pool.tile([P, NT, P], bf16, tag="wqkblk", name="wqk_blk")
                nc.sync.dma_start(out=blk, in_=w_dram[et])
                blks.append(blk)
            wqk_blks[et] = blks

        prefetch_wqk(0)
        nc.sync.dma_start(out=XT, in_=xt_d)
        prefetch_wqk(1)
        nc.gpsimd.dma_start(out=wv_blk, in_=wv)
        nc.gpsimd.dma_start(out=wo_r, in_=wo)

        # ---- QK projection units for one et: 32 matmuls + 4 bias copies ----
        def qk_proj_units(et):
            units = []
            blk_q, blk_k = wqk_blks.pop(et)
            for wqk_blk, b_sb, dst in ((blk_q, bq_sb, QT), (blk_k, bk_sb, KT)):
                for lc in range(2):
                    ps = pp.tile([P, 512], f32, tag="pp")

                    def mm(ct, ps=ps, wqk_blk=wqk_blk, lc=lc):
                        nc.tensor.matmul(
                            ps,
                            wqk_blk[:, ct, :],
                            XT[:, ct, lc * 512 : (lc + 1) * 512],
                            start=(ct == 0),
                            stop=(ct == NT - 1),
                        )

                    for ct in range(NT):
                        units.append(lambda ct=ct, mm=mm: mm(ct))

                    def bias(ps=ps, dst=dst, b_sb=b_sb, lc=lc):
                        nc.vector.tensor_scalar(
                            out=dst[:, et, lc * 512 : (lc + 1) * 512],
                            in0=ps,
                            scalar1=b_sb[:, et : et + 1],
                            scalar2=None,
                            op0=OP.add,
                        )

                    units.append(bias)
            return units

        # ---- Phase A: QK projection for et=0 (nothing to interleave) ----
        for u in qk_proj_units(0):
            u()

        # ---- Phase B: V = x @ Wv + bv ----
        for ec in range(2):
            for lt in range(NT):
                ps = pp.tile([P, 512], f32, tag="pp")
                for ct in range(NT):
                    nc.tensor.matmul(
                        ps,
                        XT[:, ct, lt * P : (lt + 1) * P],
                        wv_blk[:, ct, ec * 512 : (ec + 1) * 512],
                        start=(ct == 0),
                        stop=(ct == NT - 1),
                    )
                nc.vector.tensor_tensor(
                    out=V[:, lt, ec * 8 : (ec + 1) * 8, 0:D],
                    in0=ps.rearrange("p (h d) -> p h d", h=8),
                    in1=bv_bc[:, ec * 512 : (ec + 1) * 512].rearrange(
                        "p (h d) -> p h d", h=8
                    ),
                    op=OP.add,
                )

        # ---- Phase C: attention pair (2et, 2et+1) + QK proj for et+1 ----
        # Steps: qc=0 over kt 0..3, then qc=1 over kt 0..7.  Chunk columns
        # [lo, hi) of q; scores for the two heads issue back-to-back into
        # disjoint PE row halves (concurrent).  AV accumulates into per-qc
        # yu tiles; software-pipelined one step ahead of scores.
        steps = [(0, kt) for kt in range(4)] + [(1, kt) for kt in range(NT)]

        for et in range(NT):
            h0 = 2 * et
            last_pair = et == NT - 1
            if not last_pair and et + 2 <= NT - 1:
                prefetch_wqk(et + 2)
            units = qk_proj_units(et + 1) if not last_pair else []
            ui = 0

            st_t = {}
            pt_t = {}
            yu = {}

            def emit_scores(s):
                qc, kt = steps[s]
                lo = max(qc * 512, kt * P)
                hi = (qc + 1) * 512
                n = hi - lo
                for hh in range(2):
                    pb = hh * D
                    st = sp.tile([P, 512], f32, tag="st", name="st")
                    st_t[(s, hh)] = st
                    nc.tensor.matmul(
                        st[:, 0:n],
                        KT[pb : pb + D, et, kt * P : (kt + 1) * P],
                        QT[pb : pb + D, et, lo:hi],
                        start=True,
                        stop=True,
                    )

            def emit_exp(s):
                qc, kt = steps[s]
                lo = max(qc * 512, kt * P)
                n = (qc + 1) * 512 - lo
                diag = lo == kt * P
                for hh in range(2):
                    st = st_t.pop((s, hh))
                    pt = pt_pool.tile([P, 512], bf16, tag="pt", name="pt")
                    pt_t[(s, hh)] = pt
                    nc.scalar.activation(
                        out=pt[:, 0:n], in_=st[:, 0:n], func=AF.Exp,
                        scale=float(SCALE),
                    )
                    if diag:
                        nc.vector.tensor_tensor(
                            out=pt[:, 0:P], in0=pt[:, 0:P], in1=mask01,
                            op=OP.mult,
                        )

            def emit_av(s):
                qc, kt = steps[s]
                lo = max(qc * 512, kt * P)
                hi = (qc + 1) * 512
                n = hi - lo
                last_kt = 4 * (qc + 1) - 1
                for hh in range(2):
                    pt = pt_t.pop((s, hh))
                    if kt == 0:
                        yu[(hh, qc)] = yp.tile(
                            [D + 1, 512], f32, tag="yu", name=f"yu{qc}"
                        )
                    nc.tensor.matmul(
                        yu[(hh, qc)][:, lo - qc * 512 : hi - qc * 512],
                        V[:, kt, h0 + hh, :],
                        pt[:, 0:n],
                        start=(kt == 0),
                        stop=(kt == last_kt),
                    )
                    if kt == last_kt:
                        _head_tail(
                            nc, h0 + hh, qc, yu[(hh, qc)], Ybc, s_dram,
                            sst_pool, last_pair, ones_t, rh_pool, pp,
                        )

            emit_scores(0)
            emit_exp(0)
            for s in range(len(steps)):
                if s + 1 < len(steps):
                    emit_scores(s + 1)
                    emit_exp(s + 1)
                for _ in range(3):
                    if ui < len(units):
                        units[ui]()
                        ui += 1
                emit_av(s)
            while ui < len(units):
                units[ui]()
                ui += 1

            # softmax denominator divide for this pair (non-last pairs):
            # broadcast-read the saved row sums from DRAM on the gpsimd
            # queue, reciprocal + multiply on VectorE (overlaps next pair).
            if not last_pair:
                for half in range(2):
                    hh = h0 + half
                    bsrc = bass.AP(
                        tensor=s_dram.tensor,
                        offset=s_dram[hh : hh + 1, :].offset,
                        ap=[[0, 64], [1, L]],
                    )
                    nc.gpsimd.dma_start(
                        out=R[half * 64 : (half + 1) * 64, et, :], in_=bsrc
                    )
                nc.vector.reciprocal_approx_fast(out=R[:, et, :], in_=R[:, et, :])
                for half in range(2):
                    rows = slice(half * 64, (half + 1) * 64)
                    nc.vector.tensor_tensor(
                        out=Ybc[rows, et, :],
                        in0=Y[rows, et, :],
                        in1=R[rows, et, :],
                        op=OP.mult,
                    )

        # ---- Phase D: out = Y^T.T @ Wo + bo ----
        for lt in range(NT):
            for oc in range(2):
                ps = pp.tile([P, 512], f32, tag="pp", name="ps_out")
                for et in range(NT):
                    nc.tensor.matmul(
                        ps,
                        Ybc[:, et, lt * P : (lt + 1) * P],
                        wo_r[:, et, oc * 512 : (oc + 1) * 512],
                        start=(et == 0),
                        stop=(et == NT - 1),
                    )
                osb = osb_pool.tile([P, 512], f32)
                nc.vector.tensor_tensor(
                    out=osb, in0=ps, in1=bo_bc[:, oc * 512 : (oc + 1) * 512], op=OP.add
                )
                nc.sync.dma_start(
                    out=out[lt * P : (lt + 1) * P, oc * 512 : (oc + 1) * 512], in_=osb
                )


def _head_tail(nc, h, qc, yu_t, Ybc, s_dram, sst_pool, last_pair, ones_t, rh_pool, pp):
    et = h // 2
    pb = (h % 2) * D
    cols = slice(qc * 512, (qc + 1) * 512)
    if last_pair:
        sstb = sst_pool.tile([D + 1, 512], bf16, tag="sstb", name="sstb")
        nc.vector.tensor_copy(out=sstb[D : D + 1, :], in_=yu_t[D : D + 1, :])
        ps_bc = pp.tile([P, 512], f32, tag="pp", name="ps_bc")
        nc.tensor.matmul(
            ps_bc, ones_t[D : D + 1, :], sstb[D : D + 1, :],
            start=True, stop=True,
        )
        rh = rh_pool.tile([P, 512], f32, tag="rh", name="rh")
        nc.vector.reciprocal_approx_fast(out=rh, in_=ps_bc)
        nc.vector.tensor_tensor(
            out=Ybc[pb : pb + D, et, cols],
            in0=yu_t[0:D, :],
            in1=rh[0:D, :],
            op=OP.mult,
        )
    else:
        sst = sst_pool.tile([D + 1, 512], f32, tag="sst", name="sst")
        nc.vector.tensor_copy(out=sst[D : D + 1, :], in_=yu_t[D : D + 1, :])
        nc.gpsimd.dma_start(out=s_dram[h : h + 1, cols], in_=sst[D : D + 1, :])
        nc.vector.tensor_copy(out=Ybc[pb : pb + D, et, cols], in_=yu_t[0:D, :])


_COMPILED = None


def _get_compiled():
    global _COMPILED
    if _COMPILED is None:
        nc = _build()
        nc.compile()
        _COMPILED = nc
    return _COMPILED


def kernel(x, Wq, bq, Wk, bk, Wv, bv, Wo, bo, _trace=False):
    import ml_dtypes

    bfl = ml_dtypes.bfloat16
    nc = _get_compiled()
    x = np.ascontiguousarray(np.asarray(x, dtype=np.float32).astype(bfl))
    B = x.shape[0]
    assert B == 8 and x.shape[1] == L and x.shape[2] == E
    def _qk_layout(w):
        # [et, p, ct, e']: per-et contiguous [128, 8, 128] stationary blocks
        w = np.asarray(w, np.float32).astype(bfl)
        return np.ascontiguousarray(
            w.reshape(NT, P, NT, P).transpose(2, 1, 0, 3)
        )

    def _pct_layout(w):
        # [p, ct, e]: moving-operand blocks with contraction rows on partitions
        w = np.asarray(w, np.float32).astype(bfl)
        return np.ascontiguousarray(w.reshape(NT, P, E).transpose(1, 0, 2))

    common = {
        "wq": _qk_layout(Wq),
        "wk": _qk_layout(Wk),
        "wv": _pct_layout(Wv),
        "wo": _pct_layout(Wo),
        "bq": np.ascontiguousarray(np.asarray(bq, np.float32)),
        "bk": np.ascontiguousarray(np.asarray(bk, np.float32)),
        "bv": np.ascontiguousarray(np.asarray(bv, np.float32)),
        "bo": np.ascontiguousarray(np.asarray(bo, np.float32)),
    }
    common["mask01"] = np.tril(np.ones((P, P), np.float32)).T.astype(bfl)
    # xt[b]: [p, ct, l] with xt[b][p, ct, l] = x[b, l, ct*128+p]
    xt = np.ascontiguousarray(
        x.transpose(0, 2, 1).reshape(B, NT, P, L).transpose(0, 2, 1, 3)
    )
    in_maps = [dict(common, x=x[i], xt=xt[i]) for i in range(B)]
    res = run_bass_kernel_spmd(nc, in_maps, core_ids=list(range(8)), trace=_trace)
    outp = np.stack([res.results[i]["out"] for i in range(B)])
    if _trace:
        kernel.last_exec_time_ns = res.exec_time_ns
    return outp


# revision 5
# speedup vs baseline: 1.0222x; 1.0171x over previous
"""Causal self-attention kernel for 8 TRN2 NeuronCores.

Sharding: data-parallel over batch (B=8 -> 1 batch element per core).
Each core computes full 16-head causal attention for its batch element.
All matmuls run in bf16 with fp32 PSUM accumulation.

Per-core dataflow (L=1024, E=1024, H=16, D=64):
  XT  = x^T           host-pre-transposed bf16, one contiguous DMA
  QT  = Wq^T x^T + bq   [e, l] layout (stationary Wq blocks, moving XT)
  KT  = Wk^T x^T + bk   [e, l] layout
  V   = (x Wv + bv)|1   [l, h, 65] layout; the ones column makes the
                        attention matmul emit softmax denominators for free
  S^T = K Q^T           per (head, qc, kt): [k=128, chunk<=512] PSUM tiles,
                        contraction d=64; heads 2et/2et+1 issued back-to-back
                        as PE row-tiles (partitions 0-63 / 64-127) so the two
                        64-contraction matmuls execute concurrently
  P^T = exp(S^T*scale)  ScalarE exp -> bf16; diagonal chunk masked by a
                        0/1 multiply on VectorE
  Yu  = [V|1]^T P^T     accumulated per qc phase over kt; row 64 = softmax
                        denominator s
  Y   = Yu[0:64] / s    s broadcast via DRAM round-trip (gpsimd queue) +
                        approx reciprocal; last head pair uses a PE
                        ones-matmul broadcast instead to kill tail latency
  out = Y^T.T Wo + bo   accumulation over e-tiles, et=7 last per PSUM group

QK-projection matmuls for et+1 are interleaved between attention steps of
head pair (2et, 2et+1) so ScalarE exp stays off the TensorE critical path.
Attention is qc-phased (q columns 0-511 fully, then 512-1023) so only two
yu PSUM banks are live at a time, freeing banks for double-buffered
row-tiled score pairs.
"""

import os
import sys

sys.path.insert(0, "/opt/trn_rl_repo")

import numpy as np

import concourse.bass as bass
import concourse.mybir as mybir
import concourse.tile as tile
from concourse import bacc
from concourse.bass_utils import run_bass_kernel_spmd
f32 = mybir.dt.float32
f32r = mybir.dt.float32r
bf16 = mybir.dt.bfloat16
AF = mybir.ActivationFunctionType
OP = mybir.AluOpType

L = 1024
E = 1024
H = 16
D = 64
P = 128
NT = L // P  # 8 tiles along any 1024 dim
SCALE = 1.0 / np.sqrt(D)


def _build():
    nc = bacc.Bacc("TRN2", target_bir_lowering=False, debug=False, num_devices=8)
    x = nc.dram_tensor("x", [L, E], bf16, kind="ExternalInput").ap()
    wq = nc.dram_tensor("wq", [NT, P, NT, P], bf16, kind="ExternalInput").ap()
    wk = nc.dram_tensor("wk", [NT, P, NT, P], bf16, kind="ExternalInput").ap()
    wv = nc.dram_tensor("wv", [P, NT, E], bf16, kind="ExternalInput").ap()
    wo = nc.dram_tensor("wo", [P, NT, E], bf16, kind="ExternalInput").ap()
    bq = nc.dram_tensor("bq", [E], f32, kind="ExternalInput").ap()
    bk = nc.dram_tensor("bk", [E], f32, kind="ExternalInput").ap()
    bv = nc.dram_tensor("bv", [E], f32, kind="ExternalInput").ap()
    bo = nc.dram_tensor("bo", [E], f32, kind="ExternalInput").ap()
    xt_d = nc.dram_tensor("xt", [P, NT, L], bf16, kind="ExternalInput").ap()
    mask_d = nc.dram_tensor("mask01", [P, P], bf16, kind="ExternalInput").ap()
    out = nc.dram_tensor("out", [L, E], f32, kind="ExternalOutput").ap()
    s_dram = nc.dram_tensor("s_scratch", [H, L], f32, kind="Internal").ap()

    with tile.TileContext(nc) as tc:
        _body(nc, tc, wq, wk, wv, wo, bq, bk, bv, bo, out, s_dram,
              xt_d, mask_d)
    return nc


def _body(nc, tc, wq, wk, wv, wo, bq, bk, bv, bo, out, s_dram, xt_d, mask_d):
    from contextlib import ExitStack

    ctx = ExitStack()
    with ctx:
        consts = ctx.enter_context(tc.tile_pool(name="consts", bufs=1))
        qt_pool = ctx.enter_context(tc.tile_pool(name="qt_pool", bufs=1))
        kt_pool = ctx.enter_context(tc.tile_pool(name="kt_pool", bufs=1))
        v_pool = ctx.enter_context(tc.tile_pool(name="v_pool", bufs=1))
        y_pool = ctx.enter_context(tc.tile_pool(name="y_pool", bufs=1))
        xt_pool = ctx.enter_context(tc.tile_pool(name="xt_pool", bufs=1))
        wv_pool = ctx.enter_context(tc.tile_pool(name="wv_pool", bufs=1))
        wo_pool = ctx.enter_context(tc.tile_pool(name="wo_pool", bufs=1))
        r_pool = ctx.enter_context(tc.tile_pool(name="r_pool", bufs=1))
        sst_pool = ctx.enter_context(tc.tile_pool(name="sst_pool", bufs=4))
        rh_pool = ctx.enter_context(tc.tile_pool(name="rh_pool", bufs=3))
        wblk_pool = ctx.enter_context(tc.tile_pool(name="wblk_pool", bufs=4))
        pt_pool = ctx.enter_context(tc.tile_pool(name="pt_pool", bufs=6))
        osb_pool = ctx.enter_context(tc.tile_pool(name="osb_pool", bufs=3))
        pp = ctx.enter_context(tc.tile_pool(name="pp", bufs=2, space="PSUM"))
        sp = ctx.enter_context(tc.tile_pool(name="sp", bufs=4, space="PSUM"))
        yp = ctx.enter_context(tc.tile_pool(name="yp", bufs=2, space="PSUM"))

        # ---- constants (small DMAs on the scalar queue; broadcasts on gpsimd) ----
        mask01 = consts.tile([P, P], bf16)
        nc.scalar.dma_start(out=mask01, in_=mask_d)
        bq_sb = consts.tile([P, NT], f32)
        nc.scalar.dma_start(out=bq_sb, in_=bq.rearrange("(et p) -> p et", p=P))
        bk_sb = consts.tile([P, NT], f32)
        nc.scalar.dma_start(out=bk_sb, in_=bk.rearrange("(et p) -> p et", p=P))
        bv_bc = consts.tile([P, E], f32)
        nc.gpsimd.dma_start(
            out=bv_bc,
            in_=bass.AP(tensor=bv.tensor, offset=bv.offset, ap=[[0, P], [1, E]]),
        )
        bo_bc = consts.tile([P, E], f32)
        nc.gpsimd.dma_start(
            out=bo_bc,
            in_=bass.AP(tensor=bo.tensor, offset=bo.offset, ap=[[0, P], [1, E]]),
        )
        ones_t = consts.tile([D + 1, P], bf16)
        nc.vector.memset(ones_t, 0.0)
        nc.vector.memset(ones_t[D : D + 1, :], 1.0)

        QT = qt_pool.tile([P, NT, L], bf16)  # [p, et, l] = Q^T[et*128+p, l]
        KT = kt_pool.tile([P, NT, L], bf16)
        V = v_pool.tile([P, NT, H, D + 1], bf16)  # [p(l), lt, h, d | ones]
        Y = y_pool.tile([P, NT, L], bf16)  # [p, et, l] = y^T[et*128+p, l]
        Ybc = Y
        XT = xt_pool.tile([P, NT, L], bf16)  # [p, ct, l] = x^T[ct*128+p, l]
        wv_blk = wv_pool.tile([P, NT, E], bf16)
        wo_r = wo_pool.tile([P, NT, E], bf16)
        R = r_pool.tile([P, NT, L], f32)

        nc.vector.memset(V[:, :, :, D : D + 1], 1.0)

        # ---- bulk input DMAs ----
        # sync queue: wq/wk blocks (per-et prefetch) + XT (contiguous per
        # partition -> cheap descriptors). gpsimd queue: wv, wo (early!).
        wqk_blks = {}

        def prefetch_wqk(et):
            blks = []
            for w_dram in (wq, wk):
                blk = wblk_pool.tile([P, NT, P], bf16, tag="wqkblk", name="wqk_blk")
                nc.sync.dma_start(out=blk, in_=w_dram[et])
                blks.append(blk)
            wqk_blks[et] = blks

        # Priority order: everything the first matmuls need, then wv, then
        # prefetches, then wo (not needed until ~150us in).  XT is split so
        # QK-proj lc=0 can start after the first half lands.
        prefetch_wqk(0)
        nc.sync.dma_start(out=XT[:, :, 0:512], in_=xt_d[:, :, 0:512])
        nc.sync.dma_start(out=XT[:, :, 512:1024], in_=xt_d[:, :, 512:1024])
        nc.gpsimd.dma_start(out=wv_blk, in_=wv)
        prefetch_wqk(1)
        nc.gpsimd.dma_start(out=wo_r, in_=wo)

        # ---- QK projection units for one et: 32 matmuls + 4 bias copies ----
        def qk_proj_units(et, lc_major=False):
            units = []
            blk_q, blk_k = wqk_blks.pop(et)
            groups = [(blk_q, bq_sb, QT), (blk_k, bk_sb, KT)]
            order = (
                [(g, lc) for lc in range(2) for g in groups]
                if lc_major
                else [(g, lc) for g in groups for lc in range(2)]
            )
            for (wqk_blk, b_sb, dst), lc in order:
                if True:
                    ps = pp.tile([P, 512], f32, tag="pp")

                    def mm(ct, ps=ps, wqk_blk=wqk_blk, lc=lc):
                        nc.tensor.matmul(
                            ps,
                            wqk_blk[:, ct, :],
                            XT[:, ct, lc * 512 : (lc + 1) * 512],
                            start=(ct == 0),
                            stop=(ct == NT - 1),
                        )

                    for ct in range(NT):
                        units.append(lambda ct=ct, mm=mm: mm(ct))

                    def bias(ps=ps, dst=dst, b_sb=b_sb, lc=lc):
                        nc.vector.tensor_scalar(
                            out=dst[:, et, lc * 512 : (lc + 1) * 512],
                            in0=ps,
                            scalar1=b_sb[:, et : et + 1],
                            scalar2=None,
                            op0=OP.add,
                        )

                    units.append(bias)
            return units

        # ---- Phase A: QK projection for et=0 (nothing to interleave) ----
        for u in qk_proj_units(0, lc_major=True):
            u()

        # ---- Phase B: V = x @ Wv + bv ----
        for ec in range(2):
            for lt in range(NT):
                ps = pp.tile([P, 512], f32, tag="pp")
                for ct in range(NT):
                    nc.tensor.matmul(
                        ps,
                        XT[:, ct, lt * P : (lt + 1) * P],
                        wv_blk[:, ct, ec * 512 : (ec + 1) * 512],
                        start=(ct == 0),
                        stop=(ct == NT - 1),
                    )
                nc.vector.tensor_tensor(
                    out=V[:, lt, ec * 8 : (ec + 1) * 8, 0:D],
                    in0=ps.rearrange("p (h d) -> p h d", h=8),
                    in1=bv_bc[:, ec * 512 : (ec + 1) * 512].rearrange(
                        "p (h d) -> p h d", h=8
                    ),
                    op=OP.add,
                )

        # ---- Phase C: attention pair (2et, 2et+1) + QK proj for et+1 ----
        # Steps: qc=0 over kt 0..3, then qc=1 over kt 0..7.  Chunk columns
        # [lo, hi) of q; scores for the two heads issue back-to-back into
        # disjoint PE row halves (concurrent).  AV accumulates into per-qc
        # yu tiles; software-pipelined one step ahead of scores.
        steps = [(0, kt) for kt in range(4)] + [(1, kt) for kt in range(NT)]

        for et in range(NT):
            h0 = 2 * et
            last_pair = et == NT - 1
            if not last_pair and et + 2 <= NT - 1:
                prefetch_wqk(et + 2)
            units = qk_proj_units(et + 1) if not last_pair else []
            ui = 0

            st_t = {}
            pt_t = {}
            yu = {}

            def emit_scores(s):
                qc, kt = steps[s]
                lo = max(qc * 512, kt * P)
                hi = (qc + 1) * 512
                n = hi - lo
                for hh in range(2):
                    pb = hh * D
                    st = sp.tile([P, 512], f32, tag="st", name="st")
                    st_t[(s, hh)] = st
                    nc.tensor.matmul(
                        st[:, 0:n],
                        KT[pb : pb + D, et, kt * P : (kt + 1) * P],
                        QT[pb : pb + D, et, lo:hi],
                        start=True,
                        stop=True,
                    )

            def emit_exp(s):
                qc, kt = steps[s]
                lo = max(qc * 512, kt * P)
                n = (qc + 1) * 512 - lo
                diag = lo == kt * P
                for hh in range(2):
                    st = st_t.pop((s, hh))
                    pt = pt_pool.tile([P, 512], bf16, tag="pt", name="pt")
                    pt_t[(s, hh)] = pt
                    nc.scalar.activation(
                        out=pt[:, 0:n], in_=st[:, 0:n], func=AF.Exp,
                        scale=float(SCALE),
                    )
                    if diag:
                        nc.vector.tensor_tensor(
                            out=pt[:, 0:P], in0=pt[:, 0:P], in1=mask01,
                            op=OP.mult,
                        )

            def emit_av(s):
                qc, kt = steps[s]
                lo = max(qc * 512, kt * P)
                hi = (qc + 1) * 512
                n = hi - lo
                last_kt = 4 * (qc + 1) - 1
                for hh in range(2):
                    pt = pt_t.pop((s, hh))
                    if kt == 0:
                        yu[(hh, qc)] = yp.tile(
                            [D + 1, 512], f32, tag="yu", name=f"yu{qc}"
                        )
                    nc.tensor.matmul(
                        yu[(hh, qc)][:, lo - qc * 512 : hi - qc * 512],
                        V[:, kt, h0 + hh, :],
                        pt[:, 0:n],
                        start=(kt == 0),
                        stop=(kt == last_kt),
                    )
                    if kt == last_kt:
                        _head_tail(
                            nc, h0 + hh, qc, yu[(hh, qc)], Ybc, s_dram,
                            sst_pool, last_pair, ones_t, rh_pool, pp,
                        )

            emit_scores(0)
            emit_exp(0)
            for s in range(len(steps)):
                if s + 1 < len(steps):
                    emit_scores(s + 1)
                    emit_exp(s + 1)
                for _ in range(3):
                    if ui < len(units):
                        units[ui]()
                        ui += 1
                emit_av(s)
            while ui < len(units):
                units[ui]()
                ui += 1

            # softmax denominator divide for this pair (non-last pairs):
            # broadcast-read the saved row sums from DRAM on the gpsimd
            # queue, reciprocal + multiply on VectorE (overlaps next pair).
            if not last_pair:
                for half in range(2):
                    hh = h0 + half
                    bsrc = bass.AP(
                        tensor=s_dram.tensor,
                        offset=s_dram[hh : hh + 1, :].offset,
                        ap=[[0, 64], [1, L]],
                    )
                    nc.gpsimd.dma_start(
                        out=R[half * 64 : (half + 1) * 64, et, :], in_=bsrc
                    )
                nc.vector.reciprocal_approx_fast(out=R[:, et, :], in_=R[:, et, :])
                for half in range(2):
                    rows = slice(half * 64, (half + 1) * 64)
                    nc.vector.tensor_tensor(
                        out=Ybc[rows, et, :],
                        in0=Y[rows, et, :],
                        in1=R[rows, et, :],
                        op=OP.mult,
                    )

        # ---- Phase D: out = Y^T.T @ Wo + bo ----
        for lt in range(NT):
            for oc in range(2):
                ps = pp.tile([P, 512], f32, tag="pp", name="ps_out")
                for et in range(NT):
                    nc.tensor.matmul(
                        ps,
                        Ybc[:, et, lt * P : (lt + 1) * P],
                        wo_r[:, et, oc * 512 : (oc + 1) * 512],
                        start=(et == 0),
                        stop=(et == NT - 1),
                    )
                osb = osb_pool.tile([P, 512], f32)
                nc.vector.tensor_tensor(
                    out=osb, in0=ps, in1=bo_bc[:, oc * 512 : (oc + 1) * 512], op=OP.add
                )
                nc.sync.dma_start(
                    out=out[lt * P : (lt + 1) * P, oc * 512 : (oc + 1) * 512], in_=osb
                )


def _head_tail(nc, h, qc, yu_t, Ybc, s_dram, sst_pool, last_pair, ones_t, rh_pool, pp):
    et = h // 2
    pb = (h % 2) * D
    cols = slice(qc * 512, (qc + 1) * 512)
    if last_pair:
        sstb = sst_pool.tile([D + 1, 512], bf16, tag="sstb", name="sstb")
        nc.vector.tensor_copy(out=sstb[D : D + 1, :], in_=yu_t[D : D + 1, :])
        ps_bc = pp.tile([P, 512], f32, tag="pp", name="ps_bc")
        nc.tensor.matmul(
            ps_bc, ones_t[D : D + 1, :], sstb[D : D + 1, :],
            start=True, stop=True,
        )
        rh = rh_pool.tile([P, 512], f32, tag="rh", name="rh")
        nc.vector.reciprocal_approx_fast(out=rh, in_=ps_bc)
        nc.vector.tensor_tensor(
            out=Ybc[pb : pb + D, et, cols],
            in0=yu_t[0:D, :],
            in1=rh[0:D, :],
            op=OP.mult,
        )
    else:
        sst = sst_pool.tile([D + 1, 512], f32, tag="sst", name="sst")
        nc.vector.tensor_copy(out=sst[D : D + 1, :], in_=yu_t[D : D + 1, :])
        nc.gpsimd.dma_start(out=s_dram[h : h + 1, cols], in_=sst[D : D + 1, :])
        nc.vector.tensor_copy(out=Ybc[pb : pb + D, et, cols], in_=yu_t[0:D, :])


_COMPILED = None


def _get_compiled():
    global _COMPILED
    if _COMPILED is None:
        nc = _build()
        nc.compile()
        _COMPILED = nc
    return _COMPILED


def kernel(x, Wq, bq, Wk, bk, Wv, bv, Wo, bo, _trace=False):
    import ml_dtypes

    bfl = ml_dtypes.bfloat16
    nc = _get_compiled()
    x = np.ascontiguousarray(np.asarray(x, dtype=np.float32).astype(bfl))
    B = x.shape[0]
    assert B == 8 and x.shape[1] == L and x.shape[2] == E
    def _qk_layout(w):
        # [et, p, ct, e']: per-et contiguous [128, 8, 128] stationary blocks
        w = np.asarray(w, np.float32).astype(bfl)
        return np.ascontiguousarray(
            w.reshape(NT, P, NT, P).transpose(2, 1, 0, 3)
        )

    def _pct_layout(w):
        # [p, ct, e]: moving-operand blocks with contraction rows on partitions
        w = np.asarray(w, np.float32).astype(bfl)
        return np.ascontiguousarray(w.reshape(NT, P, E).transpose(1, 0, 2))

    common = {
        "wq": _qk_layout(Wq),
        "wk": _qk_layout(Wk),
        "wv": _pct_layout(Wv),
        "wo": _pct_layout(Wo),
        "bq": np.ascontiguousarray(np.asarray(bq, np.float32)),
        "bk": np.ascontiguousarray(np.asarray(bk, np.float32)),
        "bv": np.ascontiguousarray(np.asarray(bv, np.float32)),
        "bo": np.ascontiguousarray(np.asarray(bo, np.float32)),
    }
    common["mask01"] = np.tril(np.ones((P, P), np.float32)).T.astype(bfl)
    # xt[b]: [p, ct, l] with xt[b][p, ct, l] = x[b, l, ct*128+p]
    xt = np.ascontiguousarray(
        x.transpose(0, 2, 1).reshape(B, NT, P, L).transpose(0, 2, 1, 3)
    )
    in_maps = [dict(common, x=x[i], xt=xt[i]) for i in range(B)]
    res = run_bass_kernel_spmd(nc, in_maps, core_ids=list(range(8)), trace=_trace)
    outp = np.stack([res.results[i]["out"] for i in range(B)])
    if _trace:
        kernel.last_exec_time_ns = res.exec_time_ns
    return outp


# revision 9
# speedup vs baseline: 1.0703x; 1.0470x over previous
"""Causal self-attention kernel for 8 TRN2 NeuronCores.

Sharding: data-parallel over batch (B=8 -> 1 batch element per core).
Each core computes full 16-head causal attention for its batch element.
All matmuls run in bf16 with fp32 PSUM accumulation.

Per-core dataflow (L=1024, E=1024, H=16, D=64):
  XT  = x^T           host-pre-transposed bf16, one contiguous DMA
  QT  = Wq^T x^T + bq   [e, l] layout (stationary Wq blocks, moving XT)
  KT  = Wk^T x^T + bk   [e, l] layout
  V   = (x Wv + bv)|1   [l, h, 65] layout; the ones column makes the
                        attention matmul emit softmax denominators for free
  S^T = K Q^T           per (head, qc, kt): [k=128, chunk<=512] PSUM tiles,
                        contraction d=64; heads 2et/2et+1 issued back-to-back
                        as PE row-tiles (partitions 0-63 / 64-127) so the two
                        64-contraction matmuls execute concurrently
  P^T = exp(S^T*scale)  ScalarE exp -> bf16; diagonal chunk masked by a
                        0/1 multiply on VectorE
  Yu  = [V|1]^T P^T     accumulated per qc phase over kt; row 64 = softmax
                        denominator s
  Y   = Yu[0:64] / s    s broadcast via DRAM round-trip (gpsimd queue) +
                        approx reciprocal; last head pair uses a PE
                        ones-matmul broadcast instead to kill tail latency
  out = Y^T.T Wo + bo   accumulation over e-tiles, et=7 last per PSUM group

QK-projection matmuls for et+1 are interleaved between attention steps of
head pair (2et, 2et+1) so ScalarE exp stays off the TensorE critical path.
Attention is qc-phased (q columns 0-511 fully, then 512-1023) so only two
yu PSUM banks are live at a time, freeing banks for double-buffered
row-tiled score pairs.
"""

import os
import sys

sys.path.insert(0, "/opt/trn_rl_repo")

import numpy as np

import concourse.bass as bass
import concourse.mybir as mybir
import concourse.tile as tile
from concourse import bacc
from concourse.bass_utils import run_bass_kernel_spmd
f32 = mybir.dt.float32
f32r = mybir.dt.float32r
bf16 = mybir.dt.bfloat16
AF = mybir.ActivationFunctionType
OP = mybir.AluOpType

L = 1024
E = 1024
H = 16
D = 64
P = 128
NT = L // P  # 8 tiles along any 1024 dim
SCALE = 1.0 / np.sqrt(D)


def _build():
    nc = bacc.Bacc("TRN2", target_bir_lowering=False, debug=False, num_devices=8)
    x = nc.dram_tensor("x", [L, E], bf16, kind="ExternalInput").ap()
    wq = nc.dram_tensor("wq", [NT, P, NT, P], bf16, kind="ExternalInput").ap()
    wk = nc.dram_tensor("wk", [NT, P, NT, P], bf16, kind="ExternalInput").ap()
    wv = nc.dram_tensor("wv", [P, NT, E], bf16, kind="ExternalInput").ap()
    wo = nc.dram_tensor("wo", [P, NT, E], bf16, kind="ExternalInput").ap()
    bq = nc.dram_tensor("bq", [E], f32, kind="ExternalInput").ap()
    bk = nc.dram_tensor("bk", [E], f32, kind="ExternalInput").ap()
    # bv/bo staged host-prebroadcast to [P, E] so the load is a plain fast
    # DMA instead of a 4-byte-element SWDGE partition-broadcast.
    bv = nc.dram_tensor("bvb", [P, E], f32, kind="ExternalInput").ap()
    bo = nc.dram_tensor("bob", [P, E], f32, kind="ExternalInput").ap()
    xt_d = nc.dram_tensor("xt", [P, NT, L], bf16, kind="ExternalInput").ap()
    mask_d = nc.dram_tensor("mask01", [P, P], bf16, kind="ExternalInput").ap()
    out = nc.dram_tensor("out", [L, E], f32, kind="ExternalOutput").ap()
    s_dram = nc.dram_tensor("s_scratch", [H, L], f32, kind="Internal").ap()

    with tile.TileContext(nc) as tc:
        _body(nc, tc, wq, wk, wv, wo, bq, bk, bv, bo, out, s_dram,
              xt_d, mask_d)
    return nc


def _body(nc, tc, wq, wk, wv, wo, bq, bk, bv, bo, out, s_dram, xt_d, mask_d):
    from contextlib import ExitStack

    ctx = ExitStack()
    with ctx:
        consts = ctx.enter_context(tc.tile_pool(name="consts", bufs=1))
        qt_pool = ctx.enter_context(tc.tile_pool(name="qt_pool", bufs=1))
        kt_pool = ctx.enter_context(tc.tile_pool(name="kt_pool", bufs=1))
        v_pool = ctx.enter_context(tc.tile_pool(name="v_pool", bufs=1))
        y_pool = ctx.enter_context(tc.tile_pool(name="y_pool", bufs=1))
        xt_pool = ctx.enter_context(tc.tile_pool(name="xt_pool", bufs=1))
        wv_pool = ctx.enter_context(tc.tile_pool(name="wv_pool", bufs=1))
        wo_pool = ctx.enter_context(tc.tile_pool(name="wo_pool", bufs=1))
        r_pool = ctx.enter_context(tc.tile_pool(name="r_pool", bufs=1))
        sst_pool = ctx.enter_context(tc.tile_pool(name="sst_pool", bufs=4))
        rh_pool = ctx.enter_context(tc.tile_pool(name="rh_pool", bufs=3))
        wblk_pool = ctx.enter_context(tc.tile_pool(name="wblk_pool", bufs=4))
        pt_pool = ctx.enter_context(tc.tile_pool(name="pt_pool", bufs=6))
        osb_pool = ctx.enter_context(tc.tile_pool(name="osb_pool", bufs=3))
        pp = ctx.enter_context(tc.tile_pool(name="pp", bufs=2, space="PSUM"))
        sp = ctx.enter_context(tc.tile_pool(name="sp", bufs=4, space="PSUM"))
        yp = ctx.enter_context(tc.tile_pool(name="yp", bufs=2, space="PSUM"))

        # ---- constants (small DMAs on the scalar queue; broadcasts on gpsimd) ----
        mask01 = consts.tile([P, P], bf16)
        nc.scalar.dma_start(out=mask01, in_=mask_d)
        bq_sb = consts.tile([P, NT], f32)
        nc.scalar.dma_start(out=bq_sb, in_=bq.rearrange("(et p) -> p et", p=P))
        bk_sb = consts.tile([P, NT], f32)
        nc.scalar.dma_start(out=bk_sb, in_=bk.rearrange("(et p) -> p et", p=P))
        bv_bc = consts.tile([P, E], f32)
        nc.scalar.dma_start(out=bv_bc, in_=bv)
        bo_bc = consts.tile([P, E], f32)
        nc.scalar.dma_start(out=bo_bc, in_=bo)
        ones_t = consts.tile([D + 1, P], bf16)
        nc.vector.memset(ones_t, 0.0)
        nc.vector.memset(ones_t[D : D + 1, :], 1.0)

        QT = qt_pool.tile([P, NT, L], bf16)  # [p, et, l] = Q^T[et*128+p, l]
        KT = kt_pool.tile([P, NT, L], bf16)
        V = v_pool.tile([P, NT, H, D + 1], bf16)  # [p(l), lt, h, d | ones]
        Y = y_pool.tile([P, NT, L], bf16)  # [p, et, l] = y^T[et*128+p, l]
        Ybc = Y
        XT = xt_pool.tile([P, NT, L], bf16)  # [p, ct, l] = x^T[ct*128+p, l]
        wv_blk = wv_pool.tile([P, NT, E], bf16)
        wo_r = wo_pool.tile([P, NT, E], bf16)
        R = r_pool.tile([P, NT, L], f32)

        nc.vector.memset(V[:, :, :, D : D + 1], 1.0)

        # ---- bulk input DMAs ----
        # sync queue: wq/wk blocks (per-et prefetch) + XT (contiguous per
        # partition -> cheap descriptors). gpsimd queue: wv, wo (early!).
        wqk_blks = {}

        def prefetch_wqk(et):
            blks = []
            for w_dram in (wq, wk):
                blk = wblk_pool.tile([P, NT, P], bf16, tag="wqkblk", name="wqk_blk")
                nc.sync.dma_start(out=blk, in_=w_dram[et])
                blks.append(blk)
            wqk_blks[et] = blks

        # Strict priority via a single queue (the DMA engines round-robin
        # across queues, so separate queues would let wv/wo starve XT).
        # XT splits along ct (keeps 8KB-contiguous per-partition chunks) so
        # QK-proj ct 0-3 matmuls start while ct 4-7 is still in flight.
        prefetch_wqk(0)
        nc.sync.dma_start(out=XT[:, 0:4, :], in_=xt_d[:, 0:4, :])
        nc.sync.dma_start(out=XT[:, 4:8, :], in_=xt_d[:, 4:8, :])
        nc.sync.dma_start(out=wv_blk, in_=wv)
        prefetch_wqk(1)
        nc.sync.dma_start(out=wo_r, in_=wo)

        # ---- QK projection units for one et: 32 matmuls + 4 bias copies ----
        def qk_proj_units(et, lc_major=False):
            units = []
            blk_q, blk_k = wqk_blks.pop(et)
            groups = [(blk_q, bq_sb, QT), (blk_k, bk_sb, KT)]
            order = (
                [(g, lc) for lc in range(2) for g in groups]
                if lc_major
                else [(g, lc) for g in groups for lc in range(2)]
            )
            for (wqk_blk, b_sb, dst), lc in order:
                if True:
                    ps = pp.tile([P, 512], f32, tag="pp")

                    def mm(ct, ps=ps, wqk_blk=wqk_blk, lc=lc):
                        nc.tensor.matmul(
                            ps,
                            wqk_blk[:, ct, :],
                            XT[:, ct, lc * 512 : (lc + 1) * 512],
                            start=(ct == 0),
                            stop=(ct == NT - 1),
                        )

                    for ct in range(NT):
                        units.append(lambda ct=ct, mm=mm: mm(ct))

                    def bias(ps=ps, dst=dst, b_sb=b_sb, lc=lc):
                        nc.vector.tensor_scalar(
                            out=dst[:, et, lc * 512 : (lc + 1) * 512],
                            in0=ps,
                            scalar1=b_sb[:, et : et + 1],
                            scalar2=None,
                            op0=OP.add,
                        )

                    units.append(bias)
            return units

        # ---- Phase A: QK projection for et=0 (nothing to interleave) ----
        for u in qk_proj_units(0, lc_major=True):
            u()

        # ---- Phase B: V = x @ Wv + bv ----
        for ec in range(2):
            for lt in range(NT):
                ps = pp.tile([P, 512], f32, tag="pp")
                for ct in range(NT):
                    nc.tensor.matmul(
                        ps,
                        XT[:, ct, lt * P : (lt + 1) * P],
                        wv_blk[:, ct, ec * 512 : (ec + 1) * 512],
                        start=(ct == 0),
                        stop=(ct == NT - 1),
                    )
                nc.vector.tensor_tensor(
                    out=V[:, lt, ec * 8 : (ec + 1) * 8, 0:D],
                    in0=ps.rearrange("p (h d) -> p h d", h=8),
                    in1=bv_bc[:, ec * 512 : (ec + 1) * 512].rearrange(
                        "p (h d) -> p h d", h=8
                    ),
                    op=OP.add,
                )

        # ---- Phase C: attention pair (2et, 2et+1) + QK proj for et+1 ----
        # Steps: qc=0 over kt 0..3, then qc=1 over kt 0..7.  Chunk columns
        # [lo, hi) of q; scores for the two heads issue back-to-back into
        # disjoint PE row halves (concurrent).  AV accumulates into per-qc
        # yu tiles; software-pipelined one step ahead of scores.
        steps = [(0, kt) for kt in range(4)] + [(1, kt) for kt in range(NT)]

        for et in range(NT):
            h0 = 2 * et
            last_pair = et == NT - 1
            if not last_pair and et + 2 <= NT - 1:
                prefetch_wqk(et + 2)
            units = qk_proj_units(et + 1) if not last_pair else []
            ui = 0

            st_t = {}
            pt_t = {}
            yu = {}

            def emit_scores(s):
                qc, kt = steps[s]
                lo = max(qc * 512, kt * P)
                hi = (qc + 1) * 512
                n = hi - lo
                for hh in range(2):
                    pb = hh * D
                    st = sp.tile([P, 512], f32, tag="st", name="st")
                    st_t[(s, hh)] = st
                    nc.tensor.matmul(
                        st[:, 0:n],
                        KT[pb : pb + D, et, kt * P : (kt + 1) * P],
                        QT[pb : pb + D, et, lo:hi],
                        start=True,
                        stop=True,
                    )

            def emit_exp(s):
                qc, kt = steps[s]
                lo = max(qc * 512, kt * P)
                n = (qc + 1) * 512 - lo
                diag = lo == kt * P
                for hh in range(2):
                    st = st_t.pop((s, hh))
                    pt = pt_pool.tile([P, 512], bf16, tag="pt", name="pt")
                    pt_t[(s, hh)] = pt
                    nc.scalar.activation(
                        out=pt[:, 0:n], in_=st[:, 0:n], func=AF.Exp,
                        scale=float(SCALE),
                    )
                    if diag:
                        nc.vector.tensor_tensor(
                            out=pt[:, 0:P], in0=pt[:, 0:P], in1=mask01,
                            op=OP.mult,
                        )

            def emit_av(s):
                qc, kt = steps[s]
                lo = max(qc * 512, kt * P)
                hi = (qc + 1) * 512
                n = hi - lo
                last_kt = 4 * (qc + 1) - 1
                for hh in range(2):
                    pt = pt_t.pop((s, hh))
                    if kt == 0:
                        yu[(hh, qc)] = yp.tile(
                            [D + 1, 512], f32, tag="yu", name=f"yu{qc}"
                        )
                    nc.tensor.matmul(
                        yu[(hh, qc)][:, lo - qc * 512 : hi - qc * 512],
                        V[:, kt, h0 + hh, :],
                        pt[:, 0:n],
                        start=(kt == 0),
                        stop=(kt == last_kt),
                    )
                    if kt == last_kt:
                        _head_tail(
                            nc, h0 + hh, qc, yu[(hh, qc)], Ybc, s_dram,
                            sst_pool, last_pair, ones_t, rh_pool, pp,
                        )

            emit_scores(0)
            emit_exp(0)
            for s in range(len(steps)):
                if s + 1 < len(steps):
                    emit_scores(s + 1)
                    emit_exp(s + 1)
                for _ in range(3):
                    if ui < len(units):
                        units[ui]()
                        ui += 1
                emit_av(s)
            while ui < len(units):
                units[ui]()
                ui += 1

            # softmax denominator divide for this pair (non-last pairs):
            # broadcast-read the saved row sums from DRAM on the gpsimd
            # queue, reciprocal + multiply on VectorE (overlaps next pair).
            if not last_pair:
                for half in range(2):
                    hh = h0 + half
                    bsrc = bass.AP(
                        tensor=s_dram.tensor,
                        offset=s_dram[hh : hh + 1, :].offset,
                        ap=[[0, 64], [1, L]],
                    )
                    nc.gpsimd.dma_start(
                        out=R[half * 64 : (half + 1) * 64, et, :], in_=bsrc
                    )
                nc.vector.reciprocal_approx_fast(out=R[:, et, :], in_=R[:, et, :])
                for half in range(2):
                    rows = slice(half * 64, (half + 1) * 64)
                    nc.vector.tensor_tensor(
                        out=Ybc[rows, et, :],
                        in0=Y[rows, et, :],
                        in1=R[rows, et, :],
                        op=OP.mult,
                    )

        # ---- Phase D: out = Y^T.T @ Wo + bo ----
        for lt in range(NT):
            for oc in range(2):
                ps = pp.tile([P, 512], f32, tag="pp", name="ps_out")
                for et in range(NT):
                    nc.tensor.matmul(
                        ps,
                        Ybc[:, et, lt * P : (lt + 1) * P],
                        wo_r[:, et, oc * 512 : (oc + 1) * 512],
                        start=(et == 0),
                        stop=(et == NT - 1),
                    )
                osb = osb_pool.tile([P, 512], f32)
                nc.vector.tensor_tensor(
                    out=osb, in0=ps, in1=bo_bc[:, oc * 512 : (oc + 1) * 512], op=OP.add
                )
                nc.sync.dma_start(
                    out=out[lt * P : (lt + 1) * P, oc * 512 : (oc + 1) * 512], in_=osb
                )


def _head_tail(nc, h, qc, yu_t, Ybc, s_dram, sst_pool, last_pair, ones_t, rh_pool, pp):
    et = h // 2
    pb = (h % 2) * D
    cols = slice(qc * 512, (qc + 1) * 512)
    if last_pair:
        sstb = sst_pool.tile([D + 1, 512], bf16, tag="sstb", name="sstb")
        nc.vector.tensor_copy(out=sstb[D : D + 1, :], in_=yu_t[D : D + 1, :])
        ps_bc = pp.tile([P, 512], f32, tag="pp", name="ps_bc")
        nc.tensor.matmul(
            ps_bc, ones_t[D : D + 1, :], sstb[D : D + 1, :],
            start=True, stop=True,
        )
        rh = rh_pool.tile([P, 512], f32, tag="rh", name="rh")
        nc.vector.reciprocal_approx_fast(out=rh, in_=ps_bc)
        nc.vector.tensor_tensor(
            out=Ybc[pb : pb + D, et, cols],
            in0=yu_t[0:D, :],
            in1=rh[0:D, :],
            op=OP.mult,
        )
    else:
        sst = sst_pool.tile([D + 1, 512], f32, tag="sst", name="sst")
        nc.vector.tensor_copy(out=sst[D : D + 1, :], in_=yu_t[D : D + 1, :])
        nc.gpsimd.dma_start(out=s_dram[h : h + 1, cols], in_=sst[D : D + 1, :])
        nc.vector.tensor_copy(out=Ybc[pb : pb + D, et, cols], in_=yu_t[0:D, :])


_COMPILED = None


def _get_compiled():
    global _COMPILED
    if _COMPILED is None:
        nc = _build()
        nc.compile()
        _COMPILED = nc
    return _COMPILED


def kernel(x, Wq, bq, Wk, bk, Wv, bv, Wo, bo, _trace=False):
    import ml_dtypes

    bfl = ml_dtypes.bfloat16
    nc = _get_compiled()
    x = np.ascontiguousarray(np.asarray(x, dtype=np.float32).astype(bfl))
    B = x.shape[0]
    assert B == 8 and x.shape[1] == L and x.shape[2] == E
    def _qk_layout(w):
        # [et, p, ct, e']: per-et contiguous [128, 8, 128] stationary blocks
        w = np.asarray(w, np.float32).astype(bfl)
        return np.ascontiguousarray(
            w.reshape(NT, P, NT, P).transpose(2, 1, 0, 3)
        )

    def _pct_layout(w):
        # [p, ct, e]: moving-operand blocks with contraction rows on partitions
        w = np.asarray(w, np.float32).astype(bfl)
        return np.ascontiguousarray(w.reshape(NT, P, E).transpose(1, 0, 2))

    common = {
        "wq": _qk_layout(Wq),
        "wk": _qk_layout(Wk),
        "wv": _pct_layout(Wv),
        "wo": _pct_layout(Wo),
        "bq": np.ascontiguousarray(np.asarray(bq, np.float32)),
        "bk": np.ascontiguousarray(np.asarray(bk, np.float32)),
        "bvb": np.ascontiguousarray(
            np.broadcast_to(np.asarray(bv, np.float32), (P, E))
        ),
        "bob": np.ascontiguousarray(
            np.broadcast_to(np.asarray(bo, np.float32), (P, E))
        ),
    }
    common["mask01"] = np.tril(np.ones((P, P), np.float32)).T.astype(bfl)
    # xt[b]: [p, ct, l] with xt[b][p, ct, l] = x[b, l, ct*128+p]
    xt = np.ascontiguousarray(
        x.transpose(0, 2, 1).reshape(B, NT, P, L).transpose(0, 2, 1, 3)
    )
    in_maps = [dict(common, x=x[i], xt=xt[i]) for i in range(B)]
    res = run_bass_kernel_spmd(nc, in_maps, core_ids=list(range(8)), trace=_trace)
    outp = np.stack([res.results[i]["out"] for i in range(B)])
    if _trace:
        kernel.last_exec_time_ns = res.exec_time_ns
    return outp


# revision 21
# speedup vs baseline: 1.0721x; 1.0017x over previous
"""Causal self-attention kernel for 8 TRN2 NeuronCores.

Sharding: data-parallel over batch (B=8 -> 1 batch element per core).
Each core computes full 16-head causal attention for its batch element.
All matmuls run in bf16 with fp32 PSUM accumulation.

Per-core dataflow (L=1024, E=1024, H=16, D=64):
  XT  = x^T           host-pre-transposed bf16, one contiguous DMA
  QT  = Wq^T x^T + bq   [e, l] layout (stationary Wq blocks, moving XT)
  KT  = Wk^T x^T + bk   [e, l] layout
  V   = (x Wv + bv)|1   [l, h, 65] layout; the ones column makes the
                        attention matmul emit softmax denominators for free
  S^T = K Q^T           per (head, qc, kt): [k=128, chunk<=512] PSUM tiles,
                        contraction d=64; heads 2et/2et+1 issued back-to-back
                        as PE row-tiles (partitions 0-63 / 64-127) so the two
                        64-contraction matmuls execute concurrently
  P^T = exp(S^T*scale)  ScalarE exp -> bf16; diagonal chunk masked by a
                        0/1 multiply on VectorE
  Yu  = [V|1]^T P^T     accumulated per qc phase over kt; row 64 = softmax
                        denominator s
  Y   = Yu[0:64] / s    s broadcast via DRAM round-trip (gpsimd queue) +
                        approx reciprocal; last head pair uses a PE
                        ones-matmul broadcast instead to kill tail latency
  out = Y^T.T Wo + bo   accumulation over e-tiles, et=7 last per PSUM group

QK-projection matmuls for et+1 are interleaved between attention steps of
head pair (2et, 2et+1) so ScalarE exp stays off the TensorE critical path.
Attention is qc-phased (q columns 0-511 fully, then 512-1023) so only two
yu PSUM banks are live at a time, freeing banks for double-buffered
row-tiled score pairs.
"""

import os
import sys

sys.path.insert(0, "/opt/trn_rl_repo")

import numpy as np

import concourse.bass as bass
import concourse.mybir as mybir
import concourse.tile as tile
from concourse import bacc
from concourse.bass_utils import run_bass_kernel_spmd
f32 = mybir.dt.float32
f32r = mybir.dt.float32r
bf16 = mybir.dt.bfloat16
AF = mybir.ActivationFunctionType
OP = mybir.AluOpType

L = 1024
E = 1024
H = 16
D = 64
P = 128
NT = L // P  # 8 tiles along any 1024 dim
SCALE = 1.0 / np.sqrt(D)


def _build():
    nc = bacc.Bacc("TRN2", target_bir_lowering=False, debug=False, num_devices=8)
    x = nc.dram_tensor("x", [L, E], bf16, kind="ExternalInput").ap()
    wq = nc.dram_tensor("wq", [NT, P, NT, P], bf16, kind="ExternalInput").ap()
    wk = nc.dram_tensor("wk", [NT, P, NT, P], bf16, kind="ExternalInput").ap()
    wv = nc.dram_tensor("wv", [P, NT, E], bf16, kind="ExternalInput").ap()
    wo = nc.dram_tensor("wo", [P, NT, E], bf16, kind="ExternalInput").ap()
    # bq/bk staged host-transposed to [P, NT] (contiguous per partition)
    bq = nc.dram_tensor("bqr", [P, NT], f32, kind="ExternalInput").ap()
    bk = nc.dram_tensor("bkr", [P, NT], f32, kind="ExternalInput").ap()
    # bv/bo staged host-prebroadcast to [P, E] so the load is a plain fast
    # DMA instead of a 4-byte-element SWDGE partition-broadcast.
    bv = nc.dram_tensor("bvb", [P, E], f32, kind="ExternalInput").ap()
    bo = nc.dram_tensor("bob", [P, E], f32, kind="ExternalInput").ap()
    xt_d = nc.dram_tensor("xt", [P, NT, L], bf16, kind="ExternalInput").ap()
    mask_d = nc.dram_tensor("mask01", [P, P], bf16, kind="ExternalInput").ap()
    out = nc.dram_tensor("out", [L, E], f32, kind="ExternalOutput").ap()
    s_dram = nc.dram_tensor("s_scratch", [H, L], f32, kind="Internal").ap()

    with tile.TileContext(nc) as tc:
        _body(nc, tc, wq, wk, wv, wo, bq, bk, bv, bo, out, s_dram,
              xt_d, mask_d)
    return nc


def _body(nc, tc, wq, wk, wv, wo, bq, bk, bv, bo, out, s_dram, xt_d, mask_d):
    from contextlib import ExitStack

    ctx = ExitStack()
    with ctx:
        consts = ctx.enter_context(tc.tile_pool(name="consts", bufs=1))
        qt_pool = ctx.enter_context(tc.tile_pool(name="qt_pool", bufs=1))
        kt_pool = ctx.enter_context(tc.tile_pool(name="kt_pool", bufs=1))
        v_pool = ctx.enter_context(tc.tile_pool(name="v_pool", bufs=1))
        y_pool = ctx.enter_context(tc.tile_pool(name="y_pool", bufs=1))
        xt_pool = ctx.enter_context(tc.tile_pool(name="xt_pool", bufs=1))
        wv_pool = ctx.enter_context(tc.tile_pool(name="wv_pool", bufs=1))
        wo_pool = ctx.enter_context(tc.tile_pool(name="wo_pool", bufs=1))
        r_pool = ctx.enter_context(tc.tile_pool(name="r_pool", bufs=1))
        sst_pool = ctx.enter_context(tc.tile_pool(name="sst_pool", bufs=4))
        rh_pool = ctx.enter_context(tc.tile_pool(name="rh_pool", bufs=3))
        wblk_pool = ctx.enter_context(tc.tile_pool(name="wblk_pool", bufs=4))
        pt_pool = ctx.enter_context(tc.tile_pool(name="pt_pool", bufs=8))
        osb_pool = ctx.enter_context(tc.tile_pool(name="osb_pool", bufs=3))
        pp = ctx.enter_context(tc.tile_pool(name="pp", bufs=2, space="PSUM"))
        sp = ctx.enter_context(tc.tile_pool(name="sp", bufs=4, space="PSUM"))
        yp = ctx.enter_context(tc.tile_pool(name="yp", bufs=2, space="PSUM"))

        # ---- constants (small DMAs on the scalar queue; broadcasts on gpsimd) ----
        mask01 = consts.tile([P, P], bf16)
        nc.scalar.dma_start(out=mask01, in_=mask_d)
        bq_sb = consts.tile([P, NT], f32)
        nc.scalar.dma_start(out=bq_sb, in_=bq)
        bk_sb = consts.tile([P, NT], f32)
        nc.scalar.dma_start(out=bk_sb, in_=bk)
        bv_bc = consts.tile([P, E], f32)
        bo_bc = consts.tile([P, E], f32)
        ones_t = consts.tile([D + 1, P], bf16)
        nc.vector.memset(ones_t, 0.0)
        nc.vector.memset(ones_t[D : D + 1, :], 1.0)

        QT = qt_pool.tile([P, NT, L], bf16)  # [p, et, l] = Q^T[et*128+p, l]
        KT = kt_pool.tile([P, NT, L], bf16)
        V = v_pool.tile([P, NT, H, D + 1], bf16)  # [p(l), lt, h, d | ones]
        Y = y_pool.tile([P, NT, L], bf16)  # [p, et, l] = y^T[et*128+p, l]
        Ybc = Y
        XT = xt_pool.tile([P, NT, L], bf16)  # [p, ct, l] = x^T[ct*128+p, l]
        wv_blk = wv_pool.tile([P, NT, E], bf16)
        wo_r = wo_pool.tile([P, NT, E], bf16)
        R = r_pool.tile([P, NT, L], f32)

        nc.vector.memset(V[:, :, :, D : D + 1], 1.0)

        # ---- bulk input DMAs ----
        # sync queue: wq/wk blocks (per-et prefetch) + XT (contiguous per
        # partition -> cheap descriptors). gpsimd queue: wv, wo (early!).
        wqk_blks = {}

        def prefetch_wqk(et, eng=None):
            blks = []
            for w_dram in (wq, wk):
                blk = wblk_pool.tile([P, NT, P], bf16, tag="wqkblk", name="wqk_blk")
                (eng or nc.sync).dma_start(out=blk, in_=w_dram[et])
                blks.append(blk)
            wqk_blks[et] = blks

        # Strict priority: XT first on sync (its own queue, per-partition
        # 8KB-contiguous chunks), wq0/wk0 on scalar (parallel, small), then
        # wv -> prefetches -> wo on sync so big loads can't starve XT.
        prefetch_wqk(0, eng=nc.scalar)
        nc.sync.dma_start(out=XT[:, 0:4, :], in_=xt_d[:, 0:4, :])
        nc.sync.dma_start(out=XT[:, 4:8, :], in_=xt_d[:, 4:8, :])
        nc.sync.dma_start(out=wv_blk, in_=wv)
        prefetch_wqk(1)
        nc.sync.dma_start(out=wo_r, in_=wo)
        nc.scalar.dma_start(out=bv_bc, in_=bv)
        nc.scalar.dma_start(out=bo_bc, in_=bo)

        # ---- QK projection units for one et: 32 matmuls + 4 bias copies ----
        def qk_proj_units(et, lc_major=False):
            units = []
            blk_q, blk_k = wqk_blks.pop(et)
            groups = [(blk_q, bq_sb, QT), (blk_k, bk_sb, KT)]
            order = (
                [(g, lc) for lc in range(2) for g in groups]
                if lc_major
                else [(g, lc) for g in groups for lc in range(2)]
            )
            for (wqk_blk, b_sb, dst), lc in order:
                if True:
                    ps = pp.tile([P, 512], f32, tag="pp")

                    def mm(ct, ps=ps, wqk_blk=wqk_blk, lc=lc):
                        nc.tensor.matmul(
                            ps,
                            wqk_blk[:, ct, :],
                            XT[:, ct, lc * 512 : (lc + 1) * 512],
                            start=(ct == 0),
                            stop=(ct == NT - 1),
                        )

                    for ct in range(NT):
                        units.append(lambda ct=ct, mm=mm: mm(ct))

                    def bias(ps=ps, dst=dst, b_sb=b_sb, lc=lc):
                        nc.vector.tensor_scalar(
                            out=dst[:, et, lc * 512 : (lc + 1) * 512],
                            in0=ps,
                            scalar1=b_sb[:, et : et + 1],
                            scalar2=None,
                            op0=OP.add,
                        )

                    units.append(bias)
            return units

        # ---- Phase A: QK projection for et=0 (nothing to interleave) ----
        for u in qk_proj_units(0, lc_major=True):
            u()

        # ---- Phase B: V = x @ Wv + bv ----
        for ec in range(2):
            for lt in range(NT):
                ps = pp.tile([P, 512], f32, tag="pp")
                for ct in range(NT):
                    nc.tensor.matmul(
                        ps,
                        XT[:, ct, lt * P : (lt + 1) * P],
                        wv_blk[:, ct, ec * 512 : (ec + 1) * 512],
                        start=(ct == 0),
                        stop=(ct == NT - 1),
                    )
                nc.vector.tensor_tensor(
                    out=V[:, lt, ec * 8 : (ec + 1) * 8, 0:D],
                    in0=ps.rearrange("p (h d) -> p h d", h=8),
                    in1=bv_bc[:, ec * 512 : (ec + 1) * 512].rearrange(
                        "p (h d) -> p h d", h=8
                    ),
                    op=OP.add,
                )

        # ---- Phase C: attention pair (2et, 2et+1) + QK proj for et+1 ----
        # Steps: qc=0 over kt 0..3, then qc=1 over kt 0..7.  Chunk columns
        # [lo, hi) of q; scores for the two heads issue back-to-back into
        # disjoint PE row halves (concurrent).  AV accumulates into per-qc
        # yu tiles; software-pipelined one step ahead of scores.
        steps = [(0, kt) for kt in range(4)] + [(1, kt) for kt in range(NT)]

        # Out-proj groups whose et 0..6 partial matmuls interleave into the
        # last pair's attention (no QK proj to interleave there); their et=7
        # matmul + bias + store run first in Phase D.
        out_groups = [(lt, oc) for lt in range(NT) for oc in range(2)]
        pre_groups = out_groups[:2]
        pre_ps = {}

        def out_proj_pre_units():
            units = []
            for lt, oc in pre_groups:
                ps = pp.tile([P, 512], f32, tag="pp", name="ps_out")
                pre_ps[(lt, oc)] = ps
                for et_ in range(NT - 1):
                    def mm(ps=ps, et_=et_, lt=lt, oc=oc):
                        nc.tensor.matmul(
                            ps,
                            Ybc[:, et_, lt * P : (lt + 1) * P],
                            wo_r[:, et_, oc * 512 : (oc + 1) * 512],
                            start=(et_ == 0),
                            stop=False,
                        )
                    units.append(mm)
            return units

        for et in range(NT):
            h0 = 2 * et
            last_pair = et == NT - 1
            if not last_pair and et + 2 <= NT - 1:
                prefetch_wqk(et + 2)
            units = qk_proj_units(et + 1) if not last_pair else out_proj_pre_units()
            ui = 0

            st_t = {}
            pt_t = {}
            yu = {}

            def emit_scores(s):
                qc, kt = steps[s]
                lo = max(qc * 512, kt * P)
                hi = (qc + 1) * 512
                n = hi - lo
                for hh in range(2):
                    pb = hh * D
                    st = sp.tile([P, 512], f32, tag="st", name="st")
                    st_t[(s, hh)] = st
                    nc.tensor.matmul(
                        st[:, 0:n],
                        KT[pb : pb + D, et, kt * P : (kt + 1) * P],
                        QT[pb : pb + D, et, lo:hi],
                        start=True,
                        stop=True,
                    )

            def emit_exp(s):
                qc, kt = steps[s]
                lo = max(qc * 512, kt * P)
                n = (qc + 1) * 512 - lo
                diag = lo == kt * P
                for hh in range(2):
                    st = st_t.pop((s, hh))
                    pt = pt_pool.tile([P, 512], bf16, tag="pt", name="pt")
                    pt_t[(s, hh)] = pt
                    nc.scalar.activation(
                        out=pt[:, 0:n], in_=st[:, 0:n], func=AF.Exp,
                        scale=float(SCALE),
                    )
                    if diag:
                        nc.vector.tensor_tensor(
                            out=pt[:, 0:P], in0=pt[:, 0:P], in1=mask01,
                            op=OP.mult,
                        )

            def emit_av(s):
                qc, kt = steps[s]
                lo = max(qc * 512, kt * P)
                hi = (qc + 1) * 512
                n = hi - lo
                last_kt = 4 * (qc + 1) - 1
                for hh in range(2):
                    pt = pt_t.pop((s, hh))
                    if kt == 0:
                        yu[(hh, qc)] = yp.tile(
                            [D + 1, 512], f32, tag="yu", name=f"yu{qc}"
                        )
                    nc.tensor.matmul(
                        yu[(hh, qc)][:, lo - qc * 512 : hi - qc * 512],
                        V[:, kt, h0 + hh, :],
                        pt[:, 0:n],
                        start=(kt == 0),
                        stop=(kt == last_kt),
                    )
                    if kt == last_kt:
                        _head_tail(
                            nc, h0 + hh, qc, yu[(hh, qc)], Ybc, s_dram,
                            sst_pool, last_pair, ones_t, rh_pool, sp,
                        )

            # Two steps of score pairs issue back-to-back so the 64-row
            # LDWEIGHTS<->full-row-matmul serialization is paid once per
            # batch instead of once per step.
            nbatch = len(steps) // 2
            per_batch = (len(units) + nbatch - 1) // nbatch
            emit_scores(0)
            emit_scores(1)
            emit_exp(0)
            emit_exp(1)
            for b in range(nbatch):
                if b + 1 < nbatch:
                    emit_scores(2 * b + 2)
                    emit_scores(2 * b + 3)
                    emit_exp(2 * b + 2)
                    emit_exp(2 * b + 3)
                for _ in range(per_batch):
                    if ui < len(units):
                        units[ui]()
                        ui += 1
                emit_av(2 * b)
                emit_av(2 * b + 1)
            while ui < len(units):
                units[ui]()
                ui += 1

            # softmax denominator divide for this pair (non-last pairs):
            # broadcast-read the saved row sums from DRAM on the gpsimd
            # queue, reciprocal + multiply on VectorE (overlaps next pair).
            if not last_pair:
                for half in range(2):
                    hh = h0 + half
                    bsrc = bass.AP(
                        tensor=s_dram.tensor,
                        offset=s_dram[hh : hh + 1, :].offset,
                        ap=[[0, 64], [1, L]],
                    )
                    nc.gpsimd.dma_start(
                        out=R[half * 64 : (half + 1) * 64, et, :], in_=bsrc
                    )
                nc.vector.reciprocal_approx_fast(out=R[:, et, :], in_=R[:, et, :])
                for half in range(2):
                    rows = slice(half * 64, (half + 1) * 64)
                    nc.vector.tensor_tensor(
                        out=Ybc[rows, et, :],
                        in0=Y[rows, et, :],
                        in1=R[rows, et, :],
                        op=OP.mult,
                    )

        # ---- Phase D: out = Y^T.T @ Wo + bo ----
        def finish_group(lt, oc, ps, first_et):
            for et in range(first_et, NT):
                nc.tensor.matmul(
                    ps,
                    Ybc[:, et, lt * P : (lt + 1) * P],
                    wo_r[:, et, oc * 512 : (oc + 1) * 512],
                    start=(et == 0),
                    stop=(et == NT - 1),
                )
            osb = osb_pool.tile([P, 512], f32)
            nc.vector.tensor_tensor(
                out=osb, in0=ps, in1=bo_bc[:, oc * 512 : (oc + 1) * 512], op=OP.add
            )
            nc.sync.dma_start(
                out=out[lt * P : (lt + 1) * P, oc * 512 : (oc + 1) * 512], in_=osb
            )

        for lt, oc in pre_groups:
            finish_group(lt, oc, pre_ps[(lt, oc)], NT - 1)
        for lt, oc in out_groups:
            if (lt, oc) in pre_ps:
                continue
            ps = pp.tile([P, 512], f32, tag="pp", name="ps_out")
            finish_group(lt, oc, ps, 0)


def _head_tail(nc, h, qc, yu_t, Ybc, s_dram, sst_pool, last_pair, ones_t, rh_pool, bcp):
    et = h // 2
    pb = (h % 2) * D
    cols = slice(qc * 512, (qc + 1) * 512)
    if last_pair:
        sstb = sst_pool.tile([D + 1, 512], bf16, tag="sstb", name="sstb")
        nc.vector.tensor_copy(out=sstb[D : D + 1, :], in_=yu_t[D : D + 1, :])
        ps_bc = bcp.tile([P, 512], f32, tag="st", name="ps_bc")
        nc.tensor.matmul(
            ps_bc, ones_t[D : D + 1, :], sstb[D : D + 1, :],
            start=True, stop=True,
        )
        rh = rh_pool.tile([P, 512], f32, tag="rh", name="rh")
        nc.vector.reciprocal_approx_fast(out=rh, in_=ps_bc)
        nc.vector.tensor_tensor(
            out=Ybc[pb : pb + D, et, cols],
            in0=yu_t[0:D, :],
            in1=rh[0:D, :],
            op=OP.mult,
        )
    else:
        sst = sst_pool.tile([D + 1, 512], f32, tag="sst", name="sst")
        nc.vector.tensor_copy(out=sst[D : D + 1, :], in_=yu_t[D : D + 1, :])
        nc.gpsimd.dma_start(out=s_dram[h : h + 1, cols], in_=sst[D : D + 1, :])
        nc.vector.tensor_copy(out=Ybc[pb : pb + D, et, cols], in_=yu_t[0:D, :])


_COMPILED = None


def _get_compiled():
    global _COMPILED
    if _COMPILED is None:
        nc = _build()
        nc.compile()
        _COMPILED = nc
    return _COMPILED


def kernel(x, Wq, bq, Wk, bk, Wv, bv, Wo, bo, _trace=False):
    import ml_dtypes

    bfl = ml_dtypes.bfloat16
    nc = _get_compiled()
    x = np.ascontiguousarray(np.asarray(x, dtype=np.float32).astype(bfl))
    B = x.shape[0]
    assert B == 8 and x.shape[1] == L and x.shape[2] == E
    def _qk_layout(w):
        # [et, p, ct, e']: per-et contiguous [128, 8, 128] stationary blocks
        w = np.asarray(w, np.float32).astype(bfl)
        return np.ascontiguousarray(
            w.reshape(NT, P, NT, P).transpose(2, 1, 0, 3)
        )

    def _pct_layout(w):
        # [p, ct, e]: moving-operand blocks with contraction rows on partitions
        w = np.asarray(w, np.float32).astype(bfl)
        return np.ascontiguousarray(w.reshape(NT, P, E).transpose(1, 0, 2))

    common = {
        "wq": _qk_layout(Wq),
        "wk": _qk_layout(Wk),
        "wv": _pct_layout(Wv),
        "wo": _pct_layout(Wo),
        "bqr": np.ascontiguousarray(np.asarray(bq, np.float32).reshape(NT, P).T),
        "bkr": np.ascontiguousarray(np.asarray(bk, np.float32).reshape(NT, P).T),
        "bvb": np.ascontiguousarray(
            np.broadcast_to(np.asarray(bv, np.float32), (P, E))
        ),
        "bob": np.ascontiguousarray(
            np.broadcast_to(np.asarray(bo, np.float32), (P, E))
        ),
    }
    common["mask01"] = np.tril(np.ones((P, P), np.float32)).T.astype(bfl)
    # xt[b]: [p, ct, l] with xt[b][p, ct, l] = x[b, l, ct*128+p]
    xt = np.ascontiguousarray(
        x.transpose(0, 2, 1).reshape(B, NT, P, L).transpose(0, 2, 1, 3)
    )
    in_maps = [dict(common, x=x[i], xt=xt[i]) for i in range(B)]
    res = run_bass_kernel_spmd(nc, in_maps, core_ids=list(range(8)), trace=_trace)
    outp = np.stack([res.results[i]["out"] for i in range(B)])
    if _trace:
        kernel.last_exec_time_ns = res.exec_time_ns
    return outp
